# revision 1
# baseline (speedup 1.0000x reference)
"""DGI discriminator scores on 8 Trainium2 NeuronCores.

scores = sigmoid(einsum('bnd,de,be->bn', z, mat, s))

The einsum factors as v[b] = mat @ s[b] (tiny [512,512]x[512] contraction,
done once on-device) followed by a row-wise dot product z[b,n,:] . v[b],
so the kernel is HBM-bound on the single pass over the 204.8 MB z tensor
(~72 us at ~358 GB/s per core). Sharding: data-parallel over n — each core
gets 1/8 of the rows of both batches; mat and s are replicated.

Per core:
  z_local [2*6272, 512] f32 (batch-major, n padded 50000 -> 50176 rows).
  z is streamed in chunks of k 128-row groups (KS, ~1.75 MB DMAs, 6-deep
  buffered, alternating the two HWDGE queues); within a chunk partition p
  holds k consecutive rows. Each 128-row group is one fused
  vector-engine affine_mul_reduce (multiply by the broadcast v, reduce
  along d) -> one score column; the ISA-level tensor_tensor_reduce op
  crashes this runtime, affine_mul_reduce is the custom-DVE equivalent.
  v itself: s loaded as one [1, 1024] row and replicated across
  partitions by a K=1 ones-matmul, 4 affine_mul_reduce per batch, PE
  column transposes + another K=1 ones-matmul to replicate v. Sigmoid on the scalar engine; one [128, 98] store, decoded
  host-side. Measured steady state ~70 us/pass (HBM roofline ~72 us).
"""

import sys

import numpy as np

sys.path.insert(0, "/opt/trn_rl_repo")

B = 2
N = 50000
D = 512
N_CORES = 8
PER_CORE = 6272          # rows per batch per core (49 * 128)
NPAD = PER_CORE * N_CORES  # 50176
RG = PER_CORE // 128     # 49 row-groups of 128 rows per batch
# chunk sizes (in 128-row groups) per batch; large chunks for DMA
# efficiency, small final chunk to trim the pipeline tail
KS = [7, 7, 7, 7, 7, 7, 6, 1]
ZBUFS = 6
assert sum(KS) == RG
NCOL = B * RG            # 98 score columns

_CACHE = {}


def _build_nc(repeat=1, skip_amr=False, skip_dma=False):
    import concourse.bacc as bacc
    import concourse.bass as bass
    import concourse.mybir as mybir
    import concourse.tile as tile
    from concourse.masks import make_identity

    f32 = mybir.dt.float32
    nc = bacc.Bacc("TRN2", name="dgi_disc")
    z = nc.dram_tensor("z", [B * PER_CORE, D], f32, kind="ExternalInput")
    s = nc.dram_tensor("s", [B, D], f32, kind="ExternalInput")
    mat = nc.dram_tensor("mat", [D, D], f32, kind="ExternalInput")
    out = nc.dram_tensor("out", [128, NCOL], f32, kind="ExternalOutput")

    hwdge = [nc.sync, nc.scalar]  # two HWDGE queues, alternate for overlap

    with tile.TileContext(nc) as tc:
        with (
            tc.tile_pool(name="singles", bufs=1) as singles,
            tc.tile_pool(name="zpool", bufs=ZBUFS) as zpool,
            tc.tile_pool(name="psum", bufs=2, space=bass.MemorySpace.PSUM) as psum,
        ):
            # ---- setup: v[b] = mat @ s[b], replicated across partitions ----
            # no-dep constants first
            ident = singles.tile([128, 128], f32)
            make_identity(nc, ident)
            ones = singles.tile([1, 128], f32)
            nc.vector.memset(ones, 1.0)
            dummy = singles.tile([128, 1], f32)

            # s as one [1, 1024] row (both batches along free, partition 0),
            # then K=1 ones-matmul to replicate each 512-row across partitions
            s_row = singles.tile([1, B * D], f32)
            hwdge[0].dma_start(
                out=s_row, in_=bass.AP(tensor=s[:].tensor, offset=0, ap=[[1, 1], [1, B * D]])
            )
            s_bc = []
            for b in range(B):
                ps = psum.tile([128, D], f32, tag="sb_ps")
                nc.tensor.matmul(
                    ps, ones, s_row[0:1, b * D : (b + 1) * D], start=True, stop=True
                )
                t = singles.tile([128, D], f32, tag=f"s_bc{b}")
                nc.scalar.activation(
                    out=t, in_=ps, func=mybir.ActivationFunctionType.Copy
                )
                s_bc.append(t)
            # mat in 4 chunk DMAs so the first v reduction starts early
            mat_sb = singles.tile([128, 4, D], f32)
            for c in range(4):
                hwdge[c % 2].dma_start(
                    out=mat_sb[:, c, :], in_=mat[c * 128 : (c + 1) * 128, :]
                )

            # per batch: 4 reductions -> v on partitions -> flip -> broadcast
            v_cols = singles.tile([128, B * 4], f32)
            vrow = singles.tile([1, B * D], f32)
            v_bc = []
            for b in range(B):
                for c in range(4):
                    nc.vector.affine_mul_reduce(
                        out=dummy.broadcast_to((128, D)),
                        accum_out=v_cols[:, b * 4 + c : b * 4 + c + 1],
                        in0=mat_sb[:, c, :],
                        in1=s_bc[b],
                        scale=1.0,
                        bias=0.0,
                    )
                # flip the 4 columns to one [1, 512] psum row via PE transpose
                vr_ps = psum.tile([1, D], f32, tag="vr")
                for c in range(4):
                    nc.tensor.transpose(
                        vr_ps[0:1, c * 128 : (c + 1) * 128],
                        v_cols[:, b * 4 + c : b * 4 + c + 1],
                        ident,
                    )
                nc.scalar.activation(
                    out=vrow[0:1, b * D : (b + 1) * D],
                    in_=vr_ps,
                    func=mybir.ActivationFunctionType.Copy,
                )
                # broadcast to all 128 partitions via K=1 matmul with ones
                ps = psum.tile([128, D], f32, tag="vb")
                nc.tensor.matmul(
                    ps,
                    ones,
                    vrow[0:1, b * D : (b + 1) * D],
                    start=True,
                    stop=True,
                )
                t = singles.tile([128, D], f32, tag=f"v_bc{b}")
                nc.scalar.activation(
                    out=t, in_=ps, func=mybir.ActivationFunctionType.Copy
                )
                v_bc.append(t)

            # ---- main loop: scores[p, c*7+j] = z_row . v[b] ----
            # repeat>1 re-runs the identical loop for HW benchmarking (slope
            # between two repeat counts = steady-state time per iteration)
            scores = singles.tile([128, NCOL], f32)
            zt_static = None
            if skip_dma:
                zt_static = singles.tile([128, max(KS), D], f32, tag="zt_static")
                nc.vector.memset(zt_static, 0.5)
            # (batch, chunk row-group base, k) in issue order
            chunks = []
            for b in range(B):
                rg0 = 0
                for k in KS:
                    chunks.append((b, rg0, k))
                    rg0 += k
            for _rep in range(repeat):
                for ci, (b, rg0, k) in enumerate(chunks):
                    if skip_dma:
                        zt = zt_static
                    else:
                        zt = zpool.tile([128, max(KS), D], f32, tag="zt")
                        row0 = b * PER_CORE + rg0 * 128
                        hwdge[ci % 2].dma_start(
                            out=zt[:, :k, :],
                            in_=z[row0 : row0 + 128 * k, :].rearrange(
                                "(p k) d -> p k d", p=128
                            ),
                        )
                    if skip_amr:
                        nc.vector.tensor_copy(
                            out=scores[:, ci : ci + 1], in_=zt[:, 0, 0:1]
                        )
                        continue
                    for j in range(k):
                        col = b * RG + rg0 + j
                        nc.vector.affine_mul_reduce(
                            out=dummy.broadcast_to((128, D)),
                            accum_out=scores[:, col : col + 1],
                            in0=zt[:, j, :],
                            in1=v_bc[b],
                            scale=1.0,
                            bias=0.0,
                        )

            # sigmoid + store per batch so batch 0's output DMA overlaps
            # batch 1's remaining reductions
            sig = singles.tile([128, NCOL], f32)
            half = NCOL // B
            for b in range(B):
                cols = slice(b * half, (b + 1) * half)
                nc.scalar.activation(
                    out=sig[:, cols],
                    in_=scores[:, cols],
                    func=mybir.ActivationFunctionType.Sigmoid,
                )
                hwdge[b % 2].dma_start(out=out[:, cols], in_=sig[:, cols])

    nc.compile()
    return nc


def _get_nc():
    if "nc" not in _CACHE:
        _CACHE["nc"] = _build_nc()
    return _CACHE["nc"]


def _shard_inputs(z, s, mat):
    z = np.ascontiguousarray(z, dtype=np.float32)
    s = np.ascontiguousarray(s, dtype=np.float32)
    mat = np.ascontiguousarray(mat, dtype=np.float32)
    zp = np.zeros((B, NPAD, D), dtype=np.float32)
    zp[:, :N, :] = z
    in_maps = []
    for c in range(N_CORES):
        zc = np.ascontiguousarray(
            zp[:, c * PER_CORE : (c + 1) * PER_CORE, :]
        ).reshape(B * PER_CORE, D)
        in_maps.append({"z": zc, "s": s, "mat": mat})
    return in_maps


def _decode_core(arr):
    """[128, NCOL] device layout -> [B, PER_CORE] rows.

    Column b*RG + rg0 + j of chunk (b, rg0, k) holds rows rg0*128 + p*k + j.
    """
    local = np.empty((B, PER_CORE), dtype=np.float32)
    for b in range(B):
        rg0 = 0
        for k in KS:
            cb = b * RG + rg0
            local[b, rg0 * 128 : (rg0 + k) * 128] = arr[:, cb : cb + k].reshape(-1)
            rg0 += k
    return local


def _unshard_output(results):
    full = np.empty((B, NPAD), dtype=np.float32)
    for c in range(N_CORES):
        full[:, c * PER_CORE : (c + 1) * PER_CORE] = _decode_core(results[c]["out"])
    return np.ascontiguousarray(full[:, :N])


def kernel(z, s, mat):
    from concourse.bass_utils import run_bass_kernel_spmd

    nc = _get_nc()
    in_maps = _shard_inputs(z, s, mat)
    res = run_bass_kernel_spmd(nc, in_maps, core_ids=list(range(N_CORES)))
    return _unshard_output(res.results)


def run_traced(z, s, mat, tmpdir=None):
    """kernel() but with NTFF tracing enabled; returns (output, BassKernelResults)."""
    from concourse.bass_utils import run_bass_kernel_spmd

    nc = _get_nc()
    in_maps = _shard_inputs(z, s, mat)
    res = run_bass_kernel_spmd(
        nc, in_maps, core_ids=list(range(N_CORES)), trace=True, tmpdir=tmpdir
    )
    return _unshard_output(res.results), res

